# revision 27
# baseline (speedup 1.0000x reference)
"""Trainium2 Bass kernel for nn_ContrastiveLoss (B=4096, D=512, F=128), 8 NeuronCores.

Row-sharded: core c owns rows [c*512, (c+1)*512). All cores receive the FULL
E^T / normalized-f^T (identical buffers) plus their own 512-column weight
slices, so one static NEFF serves all cores (no per-core rolls).

Math (T=0.1 -> S' = 10*S_raw):
  bce_ij = pos ? softplus(-S') : softplus(S') = softplus(S' * sgnneg)
  softplus(x) = relu(x) + ln(1+exp(-|x|));  Sum_j relu(S'*sgnneg)
    = Sum_j relu(S') - Sum_{pos j} S'                 [relu(x)-x = relu(-x)]
  The diagonal (always pos, S'_ii ~ +5000) cancels exactly in R - P, and its
  ln-term is 0 in fp32. The dropped ln(1+exp(-|S'|)) tail is bounded by
  ln2 * #(|S_raw|<~1) per row ~ 4e1 of a ~3.7e5 row sum (~1e-4 relative).

Per core, per [128 x 1024] tile (16 tiles):
  psG = sfl^T @ sfn      (PE, 2 matmuls)   [tsim block]
  psS = etl^T @ et       (PE, 8 matmuls, PSUM-accumulated over D/128)
  ACT: relu_t = Relu(psS),            accum -> R
  mask (alternating to balance engines):
    even tiles: ACT mask_t = Sign(psG - 0.5), accum -> C  (sum of +-1)
    odd  tiles: DVE mask_t = (psG is_gt 0.5), accum -> C  (count)
  DVE: p_t = mask*psS (max(sgn,0) resp. b*1), accum -> P
Host: row_bce = 10*(R - P); pos counts from C; validity + final scalar.

All accum-pass outputs are fp32: the in-instruction accumulator follows the
OUT dtype (bf16 outs lose integer counts past 256).

This walrus build caps sync waits at 1 per instruction; _split_multiwaits
legalizes the Tile-emitted BIR by hoisting extra waits onto single-wait Drains.
"""

import json
import ml_dtypes
import numpy as np
from contextlib import ExitStack

import concourse.bass as bass
import concourse.bass_utils as bass_utils
import concourse.tile as tile
import concourse.mybir as mybir
from concourse.bass_utils import run_bass_kernel_spmd

# (walrus's --enable-ldw-opt pass rejects bass's explicit InstLdweights IR,
# so LDW dedup must happen at emission time, not in the compiler.)
# The NEFF epilogue clears every allocatable semaphore one-by-one (~9.5us for
# the default 255); this kernel uses 13, so cap the allocator's space.
import concourse.bass_utils as _bu

_orig_run_command = _bu.run_command


def _run_command_semcap(argv, **kwargs):
    if argv and "walrus_driver" in str(argv[0]):
        argv = list(argv) + ["--max-sem-num=32"]
    return _orig_run_command(argv, **kwargs)


_bu.run_command = _run_command_semcap

f32 = mybir.dt.float32
bf16 = mybir.dt.bfloat16
fp8 = mybir.dt.float8e4
AFT = mybir.ActivationFunctionType
ALU = mybir.AluOpType

B, D, F = 4096, 512, 128
NCORES = 8
RPC = B // NCORES          # 512 rows per core
NR = RPC // 128            # 4 row blocks of 128
CHUNK = 1024               # column chunk (2 PSUM banks)
NN = B // CHUNK            # 4 column chunks
NT = NR * NN               # 16 stat columns
KC = D // 128              # 4 contraction chunks
INV_T = 10.0               # 1/TEMPERATURE


def _use_sgn(idx: int) -> bool:
    """Which engine computes the tsim>0.5 mask for stat column idx.

    7 on ACT (Sign) / 9 on DVE (is_gt): ACT also runs all 16 relu passes,
    so it gets the smaller share.
    """
    return idx % 2 == 0 and idx != 14


def _dedup_ldweights(m: dict) -> int:
    """Drop PE Ldweights that reload the already-resident weights.

    bass emits one Ldweights per Matmult; with full-128-row weights the PE
    can't overlap the load with in-flight matmuls (row-group conflict), so
    each redundant load costs ~100ns of PE stream time. Consecutive
    same-weight pairs (the h=0/h=1 halves of a column chunk) need one load.
    Only PE instructions are considered; sem waits/updates on a dropped
    Ldweights are preserved on an in-place Drain.
    """
    n_rm = 0
    for fn in m["functions"]:
        for blk in fn["blocks"]:
            out = []
            last_sig = None
            for inst in blk["instructions"]:
                if inst["engine"] == "PE" and inst["opcode"] == "Ldweights":
                    sig = json.dumps(inst["ins"], sort_keys=True)
                    si = inst.get("sync_info") or {}
                    if sig == last_sig and not (si.get("on_update") or []):
                        ow = si.get("on_wait") or []
                        if ow:
                            out.append({
                                "debug": inst.get("debug", 0),
                                "engine": "PE",
                                "ins": [], "outs": [],
                                "is_reset_sema": False,
                                "name": f"{inst['name']}-ldwrm",
                                "opcode": "Drain",
                                "sync_info": {"on_update": [], "on_wait": ow},
                            })
                        n_rm += 1
                        continue
                    last_sig = sig
                out.append(inst)
            blk["instructions"] = out
    return n_rm


def _split_multiwaits(m: dict) -> int:
    """Split >1-wait instructions into single-wait Drain chains (walrus cap)."""
    n_new = 0
    for fn in m["functions"]:
        for blk in fn["blocks"]:
            out = []
            for inst in blk["instructions"]:
                si = inst.get("sync_info") or {}
                ow = si.get("on_wait") or []
                if len(ow) > 1:
                    for w in ow[:-1]:
                        n_new += 1
                        out.append({
                            "debug": inst.get("debug", 0),
                            "engine": inst["engine"],
                            "ins": [], "outs": [],
                            "is_reset_sema": False,
                            "name": f"{inst['name']}-sw{n_new}",
                            "opcode": "Drain",
                            "sync_info": {"on_update": [], "on_wait": [w]},
                        })
                    si["on_wait"] = [ow[-1]]
                out.append(inst)
            blk["instructions"] = out
    return n_new


def _build_nc() -> bass.Bass:
    nc = bass.Bass("TRN2", target_bir_lowering=False, debug=False)
    # et in fp8e4m3: halves the dominant DMA tensor; S row sums average out
    # the quantization noise (~1e-2 on row_bce). tsim stays bf16 — the
    # pos-count threshold margin (0.033) is only ~1.1x fp8's G error.
    # DoubleRow layout: [Ki=128, Ko=2, cols], contraction pair P covers
    # embedding dims [P*256, (P+1)*256) with k = P*256 + z*128 + ki.
    et_d = [nc.dram_tensor(f"et{p}", [128, 2, B], fp8,
                           kind="ExternalInput").ap() for p in range(2)]
    sfn_d = nc.dram_tensor("sfn", [F, B], bf16, kind="ExternalInput").ap()
    etl_d = [nc.dram_tensor(f"etl{p}", [128, 2, RPC], fp8,
                            kind="ExternalInput").ap() for p in range(2)]
    sfl_d = nc.dram_tensor("sfl", [F, RPC], bf16, kind="ExternalInput").ap()
    # merged stats: cols [0:NT]=relu sums, [NT:2NT]=masked sums, [2NT:3NT]=counts
    out_st = nc.dram_tensor("out_st", [128, 3 * NT], f32,
                            kind="ExternalOutput").ap()

    with tile.TileContext(nc) as tc, ExitStack() as ctx:
        main = ctx.enter_context(tc.tile_pool(name="main", bufs=1))
        scratch = ctx.enter_context(tc.tile_pool(name="scratch", bufs=3))

        etl_sb = [main.tile([128, 2, RPC], fp8, name=f"etl{p}") for p in range(2)]
        sfl_sb = main.tile([F, RPC], bf16, name="sfl_sb")
        et_sb = [main.tile([128, 2, B], fp8, name=f"et{p}") for p in range(2)]
        sfn_sb = main.tile([F, B], bf16, name="sfn_sb")

        # two dispatch rings in parallel (each DMA_DIRECT2D costs ~650ns of
        # its ring): scalar carries the G-side tensors, sync the S-side, both
        # in strict consumption order so the SDMA round-robin drains the
        # earliest-needed pieces first.
        nc.scalar.dma_start(out=sfl_sb, in_=sfl_d)
        nc.sync.dma_start(out=sfn_sb[:, 0:CHUNK], in_=sfn_d[:, 0:CHUNK])
        for p in range(2):
            nc.sync.dma_start(out=etl_sb[p], in_=etl_d[p])
            nc.sync.dma_start(out=et_sb[p][:, :, 0:CHUNK],
                              in_=et_d[p][:, :, 0:CHUNK])
        for n4 in range(1, NN):
            c0 = n4 * CHUNK
            nc.sync.dma_start(out=sfn_sb[:, c0:c0 + CHUNK],
                              in_=sfn_d[:, c0:c0 + CHUNK])
            for p in range(2):
                nc.sync.dma_start(out=et_sb[p][:, :, c0:c0 + CHUNK],
                                  in_=et_d[p][:, :, c0:c0 + CHUNK])

        neg_half = main.tile([128, 1], f32, name="neg_half")
        nc.vector.memset(neg_half, -0.5)
        ones_c = main.tile([128, CHUNK], f32, name="ones_c")
        nc.vector.memset(ones_c, 1.0)

        st = main.tile([128, 3 * NT], f32, name="st")
        r_st, p_st, c_st = st[:, 0:NT], st[:, NT:2 * NT], st[:, 2 * NT:3 * NT]

        with tc.tile_pool(name="pp_s", bufs=2, space="PSUM") as pp_s, \
             tc.tile_pool(name="pp_g", bufs=2, space="PSUM") as pp_g:
            for n4 in range(NN):
                for r in range(NR):
                    idx = n4 * NR + r
                    c0 = n4 * CHUNK
                    # G first: its cheap mask pass overlaps the S matmuls
                    psG = pp_g.tile([128, CHUNK], f32, name="psG")
                    for h in range(2):
                        nc.tensor.matmul(
                            psG[:, h * 512:(h + 1) * 512],
                            sfl_sb[:, r * 128:(r + 1) * 128],
                            sfn_sb[:, c0 + h * 512:c0 + (h + 1) * 512],
                            start=True, stop=True)
                    psS = pp_s.tile([128, CHUNK], f32, name="psS")
                    for p in range(2):
                        for h in range(2):
                            nc.tensor.matmul(
                                psS[:, h * 512:(h + 1) * 512],
                                etl_sb[p][:, :, r * 128:(r + 1) * 128],
                                et_sb[p][:, :, c0 + h * 512:c0 + (h + 1) * 512],
                                start=(p == 0), stop=(p == 1),
                                perf_mode=mybir.MatmulPerfMode.DoubleRow)

                    mask_t = scratch.tile([128, CHUNK], f32, name="mask_t")
                    if _use_sgn(idx):
                        nc.scalar.activation(mask_t, psG, AFT.Sign,
                                             bias=neg_half,
                                             accum_out=c_st[:, idx:idx + 1])
                        p_op0, p_scalar = ALU.max, 0.0
                    else:
                        # STT (1220ns) beats TensorScalar's CACHE_REDUCE
                        # lowering (1469ns) for the same mask+count
                        nc.vector.scalar_tensor_tensor(
                            out=mask_t, in0=psG, scalar=0.5, in1=ones_c,
                            op0=ALU.is_gt, op1=ALU.mult,
                            accum_out=c_st[:, idx:idx + 1])
                        p_op0, p_scalar = ALU.mult, 1.0

                    relu_t = scratch.tile([128, CHUNK], f32, name="relu_t")
                    nc.scalar.activation(relu_t, psS, AFT.Relu,
                                         accum_out=r_st[:, idx:idx + 1])
                    p_t = scratch.tile([128, CHUNK], f32, name="p_t")
                    nc.vector.scalar_tensor_tensor(
                        out=p_t, in0=mask_t, scalar=p_scalar, in1=psS,
                        op0=p_op0, op1=ALU.mult,
                        accum_out=p_st[:, idx:idx + 1])

        nc.sync.dma_start(out=out_st, in_=st)

    orig = nc.to_json_bytes

    def patched():
        m = json.loads(orig())
        _dedup_ldweights(m)
        _split_multiwaits(m)
        return json.dumps(m).encode()

    nc.to_json_bytes = patched
    return nc


_NC_CACHE = None
last_run = None  # BassKernelResults of the most recent kernel() call


def _get_nc():
    global _NC_CACHE
    if _NC_CACHE is None:
        _NC_CACHE = _build_nc()
    return _NC_CACHE


def kernel(embeddings: np.ndarray, similarity_features: np.ndarray) -> np.ndarray:
    global last_run
    E = np.asarray(embeddings, dtype=np.float32)
    SF = np.asarray(similarity_features, dtype=np.float32)
    assert E.shape == (B, D) and SF.shape == (B, F)

    ET = E.T.astype(ml_dtypes.float8_e4m3fn)                         # [D, B]
    # DoubleRow: [P, Ki, Ko=2, cols] with k = P*256 + z*128 + ki
    ET_DR = np.ascontiguousarray(ET.reshape(2, 2, 128, B).transpose(0, 2, 1, 3))
    fn = SF / np.maximum(np.linalg.norm(SF, axis=1, keepdims=True), 1e-12)
    SFN = np.ascontiguousarray(fn.T).astype(ml_dtypes.bfloat16)      # [F, B]
    in_maps = []
    for c in range(NCORES):
        sh = c * RPC
        in_maps.append({
            "et0": ET_DR[0],
            "et1": ET_DR[1],
            "sfn": SFN,
            "etl0": np.ascontiguousarray(ET_DR[0][:, :, sh:sh + RPC]),
            "etl1": np.ascontiguousarray(ET_DR[1][:, :, sh:sh + RPC]),
            "sfl": np.ascontiguousarray(SFN[:, sh:sh + RPC]),
        })

    nc = _get_nc()
    res = run_bass_kernel_spmd(nc, in_maps, core_ids=list(range(NCORES)))
    last_run = res

    # host combine: stat column idx = n4*NR + r; local row i = r*128 + p
    bce_num = np.zeros((NCORES, RPC), np.float64)
    pos_all = np.zeros((NCORES, RPC), np.float64)
    sgn_cols = np.array([_use_sgn(i) for i in range(NT)])
    for c, r in enumerate(res.results):
        def rows(a):
            # [128, NT] -> per-row sums over the NN chunks -> [RPC]
            return a.reshape(128, NN, NR).sum(axis=1).T.reshape(RPC)
        st = r["out_st"].astype(np.float64)
        R, P, cc = st[:, 0:NT], st[:, NT:2 * NT], st[:, 2 * NT:3 * NT]
        pos_chunks = np.where(sgn_cols[None, :], (CHUNK + cc) / 2.0, cc)
        bce_num[c] = INV_T * (rows(R) - rows(P))
        pos_all[c] = rows(pos_chunks)

    bce_num = bce_num.reshape(-1)
    pos_off = pos_all.reshape(-1) - 1.0    # diagonal is always a positive
    neg_off = (B - 1) - pos_off
    row_loss = bce_num / np.float64(B - 1)
    valid = (pos_off >= 0.5) & (neg_off >= 0.5)
    num_valid = max(int(valid.sum()), 1)
    loss = np.float64(np.sum(np.where(valid, row_loss, 0.0))) / num_valid
    return np.float32(loss)
